# revision 2
# baseline (speedup 1.0000x reference)
"""Trainium2 Bass kernel for nn_ChoquetIntegralConstrained.

Computes: sigmoid((x @ w_eff) / weight_sum - thr) where w_eff is built from
(wc, wint) via the constraint transform, x is [16384, 8256] f32.

Strategy: pure data parallel over batch across 8 NeuronCores. Each core gets
2048 rows, streamed as 64 chunk DMAs of [128 rows, 2064 cols] alternating the
two HWDGE rings. Each chunk is consumed by one fused DVE STT (out = x*w,
accum_out = row-sum) so the Vector engine tracks the HBM stream within one
chunk (~2.4 us). The weight vector is uploaded as a bf16 hi/lo pair and
broadcast to all 128 partitions by the (otherwise idle) TensorEngine at
full bf16 rate: ones[2,128] stationary x w2[2,N] moving -> PSUM f32 holds
hi+lo = fp32-accurate w; ACT copies PSUM->SBUF. Weights are ready ~4 us into
the kernel instead of ~41 us for an fp32 PE broadcast. The tiny constraint
transform on the 8256 weights is done on the host in fp32 (identical
elementwise semantics to the reference).
"""

import sys

import numpy as np

sys.path.insert(0, "/opt/trn_rl_repo")

N_CRIT = 128
N_PAIRS = N_CRIT * (N_CRIT - 1) // 2  # 8128
D = N_CRIT + N_PAIRS  # 8256
BATCH = 16384
N_CORES = 8
ROWS_PER_CORE = BATCH // N_CORES  # 2048
P = 128  # SBUF partitions
TILES_PER_CORE = ROWS_PER_CORE // P  # 16
CH = D // 4  # 2064: chunk width; 4 chunks per row-tile
N_CHUNKS = TILES_PER_CORE * 4  # 64
MIN_W = np.float32(1e-07)

_CACHE = {}


def _build_program():
    import concourse.tile as tile
    from concourse import bacc, mybir

    nc = bacc.Bacc(
        "TRN2",
        debug=False,
        target_bir_lowering=False,
        num_devices=N_CORES,
    )
    f32 = mybir.dt.float32
    bf16 = mybir.dt.bfloat16
    x_d = nc.dram_tensor("x", [ROWS_PER_CORE, D], f32, kind="ExternalInput").ap()
    w_d = nc.dram_tensor("w2", [2, D], bf16, kind="ExternalInput").ap()
    c_d = nc.dram_tensor("consts", [P, 2], f32, kind="ExternalInput").ap()
    y_d = nc.dram_tensor("y", [P, TILES_PER_CORE], f32, kind="ExternalOutput").ap()

    NBUF = 16  # chunk pool depth: ~39 us of DMA-ahead decoupling
    EARLY = 12  # x chunks dispatched before the PE/ACT weight block

    with tile.TileContext(nc) as tc:
        with (
            tc.tile_pool(name="xcp", bufs=NBUF) as xcp,
            tc.tile_pool(name="wp", bufs=1) as wp,
            tc.tile_pool(name="pp", bufs=2, space="PSUM") as pp,
        ):
            # Persistent tiles
            w_q0 = wp.tile([P, CH], f32)
            w_q1 = wp.tile([P, CH], f32)
            w_q2 = wp.tile([P, CH], f32)
            w_q3 = wp.tile([P, CH], f32)
            w_quarters = [w_q0, w_q1, w_q2, w_q3]
            accq_t = wp.tile([P, N_CHUNKS], f32)
            acc_t = wp.tile([P, TILES_PER_CORE], f32)
            y_t = wp.tile([P, TILES_PER_CORE], f32)
            c_t = wp.tile([P, 2], f32)
            # STT must write a full-size out; a stride-0 broadcast AP over a
            # [P, 1] dummy absorbs it without SBUF cost.
            dummy = wp.tile([P, 1], f32)

            # w2 (bf16 hi/lo pair) + consts ride the SWDGE (gpsimd) ring so
            # the two HWDGE rings carry nothing but x chunks + the y store.
            w2_t = wp.tile([2, D], bf16)
            nc.gpsimd.dma_start(out=w2_t[:], in_=w_d[:])
            ones_t = wp.tile([2, P], bf16)
            nc.gpsimd.memset(ones_t[:], 1.0)
            nc.gpsimd.dma_start(out=c_t[:], in_=c_d[:])

            # x chunk DMAs alternate between the two HWDGE rings (ACT and SP).
            dma_engines = (nc.scalar, nc.sync)
            chunk_tiles = [None] * N_CHUNKS

            def issue_chunk(g):
                t, q = divmod(g, 4)
                x_c = xcp.tile([P, CH], f32)
                chunk_tiles[g] = x_c
                rows = slice(t * P, (t + 1) * P)
                dma_engines[g % 2].dma_start(
                    out=x_c[:], in_=x_d[rows, q * CH : (q + 1) * CH]
                )

            def consume_chunk(g):
                t, q = divmod(g, 4)
                nc.vector.scalar_tensor_tensor(
                    out=dummy.broadcast_to((P, CH)),
                    in0=chunk_tiles[g][:],
                    scalar=1.0,
                    in1=w_quarters[q][:],
                    op0=mybir.AluOpType.mult,
                    op1=mybir.AluOpType.mult,
                    accum_out=accq_t[:, g : g + 1],
                )

            # Dispatch the first EARLY chunk DMAs before the weight block so
            # both HWDGE rings start streaming x at t~0 (per-engine program
            # order would otherwise park the ACT ring behind the PSUM->SBUF
            # copies below).
            for g in range(EARLY):
                issue_chunk(g)

            # Weight broadcast: ones[2,128] bf16 stationary, w2[2,N] bf16
            # moving -> PSUM[128,N] f32 = w_hi + w_lo (fp32-accurate), then
            # ACT copies PSUM->SBUF quarters. Never touches the DMA engines
            # streaming x; full bf16 PE rate.
            MMCH = 512  # one PSUM bank (2 KB / partition) per matmul
            for q in range(4):
                for j in range(0, CH, MMCH):
                    n = min(MMCH, CH - j)
                    mm = pp.tile([P, MMCH], f32)
                    nc.tensor.matmul(
                        mm[:, 0:n],
                        ones_t[:],
                        w2_t[:, q * CH + j : q * CH + j + n],
                        start=True,
                        stop=True,
                    )
                    nc.scalar.copy(w_quarters[q][:, j : j + n], mm[:, 0:n])

            # Steady state: issue chunk g+EARLY, consume chunk g.
            for g in range(N_CHUNKS):
                if g + EARLY < N_CHUNKS:
                    issue_chunk(g + EARLY)
                consume_chunk(g)

            # Combine the 4 quarter partial sums of every tile. Tiles 0..14
            # reduce as soon as their STTs are done; tile 15 reduces alone so
            # the tail after the last chunk STT is minimal.
            nc.vector.tensor_reduce(
                out=acc_t[:, 0 : TILES_PER_CORE - 1],
                in_=accq_t[:, 0 : 4 * (TILES_PER_CORE - 1)].rearrange(
                    "p (t q) -> p t q", q=4
                ),
                axis=mybir.AxisListType.X,
                op=mybir.AluOpType.add,
            )
            nc.vector.tensor_reduce(
                out=acc_t[:, TILES_PER_CORE - 1 : TILES_PER_CORE],
                in_=accq_t[:, 4 * (TILES_PER_CORE - 1) : N_CHUNKS].rearrange(
                    "p (t q) -> p t q", q=4
                ),
                axis=mybir.AxisListType.X,
                op=mybir.AluOpType.add,
            )

            nc.scalar.activation(
                out=y_t[:],
                in_=acc_t[:],
                func=mybir.ActivationFunctionType.Sigmoid,
                bias=c_t[:, 1:2],
                scale=c_t[:, 0:1],
            )
            nc.sync.dma_start(out=y_d[:], in_=y_t[:])

    nc.compile()
    return nc


def _get_program():
    if "nc" not in _CACHE:
        _CACHE["nc"] = _build_program()
    return _CACHE["nc"]


def _host_weight_prep(wc, wint, thr):
    """Mirror reference._constrained_weights + weight_sum in fp32 numpy."""
    wc = np.asarray(wc, dtype=np.float32)
    wint = np.asarray(wint, dtype=np.float32)
    wc_eff = np.where(wc < 0, MIN_W, wc)
    ii, jj = np.triu_indices(N_CRIT, k=1)
    lower = np.maximum(-wc_eff[:, ii], -wc_eff[:, jj])
    wint_eff = np.maximum(wint, lower)
    w_eff = np.concatenate([wc_eff, wint_eff], axis=1)  # [1, D]
    wsum = np.float32(wc_eff.sum(dtype=np.float32)) + np.float32(
        wint_eff.sum(dtype=np.float32)
    )
    inv_wsum = np.float32(1.0) / wsum
    neg_thr = -np.float32(np.asarray(thr).reshape(-1)[0])
    return w_eff, inv_wsum, neg_thr


def _make_in_maps(x, wc, wint, thr):
    import ml_dtypes

    x = np.ascontiguousarray(np.asarray(x, dtype=np.float32))
    w_eff, inv_wsum, neg_thr = _host_weight_prep(wc, wint, thr)
    # bf16 hi/lo split: hi + lo == w_eff to ~2^-18 relative accuracy; the PE
    # broadcast sums them in fp32 PSUM.
    w_hi = w_eff.astype(ml_dtypes.bfloat16)
    w_lo = (w_eff - w_hi.astype(np.float32)).astype(ml_dtypes.bfloat16)
    w2 = np.ascontiguousarray(np.concatenate([w_hi, w_lo], axis=0))  # [2, D]
    consts = np.empty((P, 2), dtype=np.float32)
    consts[:, 0] = inv_wsum
    consts[:, 1] = neg_thr
    return [
        {
            "x": np.ascontiguousarray(x[c * ROWS_PER_CORE : (c + 1) * ROWS_PER_CORE]),
            "w2": w2,
            "consts": consts,
        }
        for c in range(N_CORES)
    ]


def _gather(results):
    # y core tile is [P, TILES]: y[p, t] = batch row t*128 + p within the shard
    parts = [
        np.asarray(results[c]["y"]).T.reshape(ROWS_PER_CORE) for c in range(N_CORES)
    ]
    return np.concatenate(parts).reshape(BATCH, 1).astype(np.float32)


def _run(x, wc, wint, thr, trace=False):
    from concourse import bass_utils

    nc = _get_program()
    in_maps = _make_in_maps(x, wc, wint, thr)
    res = bass_utils.run_bass_kernel_spmd(
        nc, in_maps, core_ids=list(range(N_CORES)), trace=trace
    )
    return _gather(res.results), res


def kernel(x, wc, wint, thr):
    out, _ = _run(x, wc, wint, thr, trace=False)
    return out
